# revision 21
# baseline (speedup 1.0000x reference)
"""MoE (noisy top-2 routing) Trainium2 kernel.

Strategy (expert parallelism, per sharding hint):
  - Host: compute gating logits + top-2 + softmax gates (cheap: T x E),
    gather each expert's tokens into a padded [capacity, D] batch.
  - Device (8 cores, 2 experts/core): per expert FFN
        hT = relu(W1^T @ x_e^T + b1)        (fp16 in, fp32 PSUM)
        y  = gate * (hT^T @ W2)             (fp16 in, fp32 PSUM)
    Layout trick: matmul1 produces h TRANSPOSED ([H, C]) so its output tiles
    are directly the lhsT (stationary) operand of matmul2 - no transposes.
    b1 is applied for free in the ScalarE relu pass (per-partition bias),
    the gate scale for free in the ScalarE PSUM->SBUF copy (per-partition
    scale). b2's contribution (sum_k g_k * b2[e_k]) is added on host.
  - Host: combine = two gathers + add (gates already applied on device).

Expert pairing: experts are sorted by token count and paired
largest-with-smallest; slot 0 takes the larger expert. Slot capacities
(C0 >= C1) are compiled into the kernel, minimizing padded work while
keeping all 8 cores on identical shapes (SPMD).
"""

import math
from contextlib import ExitStack

import numpy as np

import concourse.bacc as bacc
import concourse.bass as bass
import concourse.mybir as mybir
import concourse.tile as tile
from concourse.bass_utils import run_bass_kernel_spmd

T, D, H, E, TOPK = 4096, 1024, 2048, 16, 2
NOISE_SCALE = 1.0
P = 128
NCORES = 8
EPC = E // NCORES  # experts per core
KD = D // P  # 8  contraction tiles for matmul1
KH = H // P  # 16 contraction tiles for matmul2
ND = D // 512  # 2  output free-dim chunks for matmul2

F16 = mybir.dt.float16
F32 = mybir.dt.float32

_CACHE: dict[tuple, bass.Bass] = {}
LAST_RESULTS = None  # BassKernelResults of the most recent run (for profiling)
TRACE = False  # set True (e.g. from test.py) to capture an NTFF trace


def _n_chunks(C: int) -> list[tuple[int, int]]:
    """Split C columns into (offset, size) chunks of <=512 (PSUM bank limit)."""
    out = []
    c0 = 0
    while c0 < C:
        n = min(512, C - c0)
        out.append((c0, n))
        c0 += n
    return out


def _build_nc(caps: tuple[int, ...]) -> bass.Bass:
    """Bass module for one core: EPC expert FFNs, expert slot e padded to
    caps[e] tokens."""
    CPs = [math.ceil(c / P) for c in caps]  # gate-scale columns per slot
    # misc layout per slot: [P, KH (b1 columns) + CP (gate columns)]
    misc_cols = sum(KH + cp for cp in CPs)

    nc = bacc.Bacc()
    # all inputs pre-swizzled on host to [P, outer, free] so DMAs are fully
    # contiguous per partition (max burst, min descriptors)
    xts_d = [
        nc.declare_dram_parameter(f"xt{e}", [P, KD, caps[e]], F16, isOutput=False)
        for e in range(EPC)
    ]
    w1 = nc.declare_dram_parameter("w1", [EPC, P, KD, H], F16, isOutput=False)
    w2 = nc.declare_dram_parameter("w2", [EPC, P, KH, D], F16, isOutput=False)
    misc = nc.declare_dram_parameter("misc", [P, misc_cols], F32, isOutput=False)
    ys_d = [
        nc.declare_dram_parameter(f"y{e}", [caps[e], D], F32, isOutput=True)
        for e in range(EPC)
    ]

    with ExitStack() as ctx:
        tc = ctx.enter_context(tile.TileContext(nc))
        xt_pool = ctx.enter_context(tc.tile_pool(name="xt_pool", bufs=2))
        w1_pool = ctx.enter_context(tc.tile_pool(name="w1_pool", bufs=2))
        w2_pool = ctx.enter_context(tc.tile_pool(name="w2_pool", bufs=2))
        h_pool = ctx.enter_context(tc.tile_pool(name="h_pool", bufs=2))
        y_pool = ctx.enter_context(tc.tile_pool(name="y_pool", bufs=4))
        c_pool = ctx.enter_context(tc.tile_pool(name="c_pool", bufs=1))
        ps1_pool = ctx.enter_context(tc.tile_pool(name="ps1_pool", bufs=2, space="PSUM"))
        ps2_pool = ctx.enter_context(tc.tile_pool(name="ps2_pool", bufs=4, space="PSUM"))

        # biases + gates: one small transfer on the GPSIMD (SWDGE) queue so it
        # never sits in front of the big weight DMAs on the sync queue.
        miscs = c_pool.tile([P, misc_cols], F32, name="miscs", tag="miscs")
        nc.gpsimd.dma_start(miscs[:], misc[:, :])
        moff = [0]
        for e in range(EPC):
            moff.append(moff[-1] + KH + CPs[e])

        # larger slot (with its small remainder tile) last: smaller final
        # output DMA on the kernel's critical tail
        for e in sorted(range(EPC), key=lambda s: caps[s]):
            C = caps[e]
            CP = CPs[e]
            nchunks = _n_chunks(C)
            nmc = math.ceil(C / P)
            b1s = miscs[:, moff[e] : moff[e] + KH]
            gs = miscs[:, moff[e] + KH : moff[e] + KH + CP]

            xts = xt_pool.tile([P, KD, C], F16, name=f"xts{e}", tag=f"xts{e}", bufs=1)
            nc.sync.dma_start(xts[:], xts_d[e][:, :, :])
            # split the W1 load so the first matmuls start before all 4MB lands
            w1s = w1_pool.tile([P, KD, H], F16, name=f"w1s{e}", tag="w1s")
            half = KD // 2
            nc.sync.dma_start(w1s[:, :half], w1[e, :, :half])
            # second half via the ScalarE HWDGE queue: parallel ring with the
            # sync queue, and this trigger carries no waits so it cannot
            # block the relu ACTs that come later on that queue
            nc.scalar.dma_start(w1s[:, half:], w1[e, :, half:])
            w2s = w2_pool.tile([P, KH, D], F16, name=f"w2s{e}", tag="w2s")
            nc.sync.dma_start(w2s[:], w2[e, :, :, :])

            # hT, fp16, [H, C] as KH tiles of [128, C]; partition = h within tile
            hts = h_pool.tile([P, KH, C], F16, name=f"hts{e}", tag=f"hts{e}", bufs=1)

            # ---- matmul1: hT[mh] = relu(sum_kd W1[kd,mh]^T.T @ xT[kd] + b1) ----
            for mh in range(KH):
                pss = [
                    ps1_pool.tile([P, n], F32, name=f"ps1_{e}_{mh}_{i}", tag=f"ps1_{i}")
                    for i, (_, n) in enumerate(nchunks)
                ]
                for kd in range(KD):
                    lhsT = w1s[:, kd, mh * P : (mh + 1) * P]
                    for i, (c0, n) in enumerate(nchunks):
                        nc.tensor.matmul(
                            pss[i][:, :],
                            lhsT=lhsT,
                            rhs=xts[:, kd, c0 : c0 + n],
                            start=(kd == 0),
                            stop=(kd == KD - 1),
                        )
                for i, (c0, n) in enumerate(nchunks):
                    nc.scalar.activation(
                        hts[:, mh, c0 : c0 + n],
                        pss[i][:, :],
                        mybir.ActivationFunctionType.Relu,
                        bias=b1s[:, mh : mh + 1],
                    )

            # ---- matmul2: y[mc] = g * (sum_kh hT[kh,mc].T @ W2[kh]) ----
            for mc in range(nmc):
                c0 = mc * P
                mcn = min(P, C - c0)
                for nd in range(ND):
                    ys = y_pool.tile([P, 512], F32, name=f"ys{e}_{mc}_{nd}", tag="ys")
                    psy = ps2_pool.tile([P, 512], F32, name=f"psy{e}_{mc}_{nd}", tag="psy")
                    for kh in range(KH):
                        nc.tensor.matmul(
                            psy[:mcn, :],
                            lhsT=hts[:, kh, c0 : c0 + mcn],
                            rhs=w2s[:, kh, nd * 512 : (nd + 1) * 512],
                            start=(kh == 0),
                            stop=(kh == KH - 1),
                        )
                    # gate scale on the (otherwise idle) vector engine
                    nc.vector.tensor_scalar_mul(
                        ys[:mcn, :], psy[:mcn, :], gs[:mcn, mc : mc + 1]
                    )
                    nc.sync.dma_start(
                        ys_d[e][c0 : c0 + mcn, nd * 512 : (nd + 1) * 512], ys[:mcn, :]
                    )

    nc.compile()
    return nc


def _route(x, noise_eps, Wg, Wn):
    """Replicate the reference noisy top-2 gating on host (fp64)."""
    xl = x.astype(np.float64)
    logits = xl @ Wg.astype(np.float64).T + NOISE_SCALE * noise_eps.astype(
        np.float64
    ) * np.logaddexp(0.0, xl @ Wn.astype(np.float64).T)
    # jax.lax.top_k: values sorted descending, ties broken by lower index
    top_idx = np.argsort(-logits, axis=1, kind="stable")[:, :TOPK]
    tv = np.take_along_axis(logits, top_idx, axis=1)
    ex = np.exp(tv - tv.max(axis=1, keepdims=True))
    gates = ex / ex.sum(axis=1, keepdims=True)
    return top_idx, gates.astype(np.float32)


def kernel(x, noise_eps, Wg, Wn, W1, b1, W2, b2):
    global LAST_RESULTS
    # inputs may arrive as jax arrays; force plain numpy so all host math
    # (routing, gather/scatter) stays off-device
    x = np.ascontiguousarray(np.asarray(x), np.float32)
    noise_eps = np.asarray(noise_eps, np.float32)
    Wg = np.asarray(Wg, np.float32)
    Wn = np.asarray(Wn, np.float32)
    W1 = np.asarray(W1, np.float32)
    b1 = np.asarray(b1, np.float32)
    W2 = np.asarray(W2, np.float32)
    b2 = np.asarray(b2, np.float32)

    top_idx, gates = _route(x, noise_eps, Wg, Wn)

    # token lists per expert
    tok_lists = [np.nonzero((top_idx == e).any(axis=1))[0] for e in range(E)]
    counts = np.array([len(t) for t in tok_lists])

    # pair largest with smallest; slot 0 = larger expert of each pair
    order = np.argsort(-counts, kind="stable")
    slot_expert = np.zeros((NCORES, EPC), np.int64)  # (core, slot) -> expert
    for c in range(NCORES):
        slot_expert[c, 0] = order[c]
        slot_expert[c, 1] = order[E - 1 - c]
    cap = lambda n: max(64, int(math.ceil(n / 64) * 64))
    caps = tuple(
        cap(int(counts[slot_expert[:, s]].max())) for s in range(EPC)
    )  # per-slot capacity, uniform across cores
    CPs = [math.ceil(c / P) for c in caps]

    nc = _CACHE.get(caps)
    if nc is None:
        nc = _CACHE[caps] = _build_nc(caps)

    x16 = x.astype(np.float16)
    W1_16 = np.asarray(W1, np.float16)
    W2_16 = np.asarray(W2, np.float16)
    b1f = np.asarray(b1, np.float32)

    # position of (t, k) within its expert's batch
    pos_of = np.zeros((T, TOPK), np.int64)
    misc_cols = sum(KH + cp for cp in CPs)

    in_maps = []
    for c in range(NCORES):
        m = {}
        misc_np = np.zeros((P, misc_cols), np.float32)
        mo = 0
        for s in range(EPC):
            e = int(slot_expert[c, s])
            C = caps[s]
            CP = CPs[s]
            toks = tok_lists[e]
            xt_np = np.zeros((KD, P, C), np.float16)
            xt_np[:, :, : len(toks)] = x16[toks].T.reshape(KD, P, -1)
            m[f"xt{s}"] = np.ascontiguousarray(xt_np.transpose(1, 0, 2))
            k_sel = (top_idx[toks] == e).argmax(axis=1)
            pos_of[toks, k_sel] = np.arange(len(toks))
            misc_np[:, mo : mo + KH] = b1f[e].reshape(KH, P).T
            g_col = np.zeros(CP * P, np.float32)
            g_col[: len(toks)] = gates[toks, k_sel]
            misc_np[:, mo + KH : mo + KH + CP] = g_col.reshape(CP, P).T
            mo += KH + CP
        m["misc"] = misc_np
        sl = slot_expert[c]
        m["w1"] = np.ascontiguousarray(
            W1_16[sl].reshape(EPC, KD, P, H).transpose(0, 2, 1, 3)
        )
        m["w2"] = np.ascontiguousarray(
            W2_16[sl].reshape(EPC, KH, P, D).transpose(0, 2, 1, 3)
        )
        in_maps.append(m)

    res = run_bass_kernel_spmd(nc, in_maps, core_ids=list(range(NCORES)), trace=TRACE)
    LAST_RESULTS = res

    # Y[e] = gate-scaled outputs of expert e, [caps[slot], D]
    Y = [None] * E
    for c in range(NCORES):
        for s in range(EPC):
            Y[int(slot_expert[c, s])] = res.results[c][f"y{s}"]

    # max capacity stack for a single vectorized gather
    Cmax = max(caps)
    Yall = np.zeros((E, Cmax, D), np.float32)
    for e in range(E):
        Yall[e, : Y[e].shape[0]] = Y[e]

    out = Yall[top_idx[:, 0], pos_of[:, 0]] + Yall[top_idx[:, 1], pos_of[:, 1]]
    b2f = np.asarray(b2, np.float32)
    out += gates[:, 0:1] * b2f[top_idx[:, 0]] + gates[:, 1:2] * b2f[top_idx[:, 1]]
    return out.astype(np.float32)
